# revision 21
# baseline (speedup 1.0000x reference)
"""PostCrossAttention Trainium2 kernel.

Reference computation (per batch b):
    qh = (q @ Wq.T)  split into H=8 heads of dh=96   -> [H, N, 96]
    kh = (k @ Wk.T)  likewise
    vh = (v @ Wv.T)  split into H=8 heads of dv=64   -> [H, N, 64]
    S  = qh @ kh.T * SCALE          (SCALE = (256//8)**-0.5 = 32**-0.5)
    A  = softmax(S, axis=-1)
    A  = A * m / (H * sum(m, -1, keepdims))
    x  = A @ vh   -> concat heads -> [N, 512]

Sharding: 8 cores = 4 batches x 2 head-groups (4 heads each).
Each core receives host-pre-transposed bf16 operands and computes its
[2048, 256] slice of the output.

Device dataflow (per core, per head):
    S.T[j,i] = Kp @ Qp.T   (via lhsT=KpT tile, rhs=QpT, K=96 contraction)
    expS.T   = exp(S.T * SCALE)            (ACT, from PSUM, bf16 out)
    B.T      = expS.T * masks.T            (DVE, bf16)
    U.T[0:64 ,i] += Vp[jt].T @ B.T[jt]     (PE, accumulated over jt)
    U.T[64:65,i] += ones.T   @ expS.T[jt]  (PE, = sumexp row)
    U = transpose(U.T)  (PE, 128x65 tiles) then
    x[i, d] = U[i, d] / (8 * summ[i] * sumexp[i])   (DVE)
"""

import sys

for _p in ("/opt/trn_rl_repo",):
    if _p not in sys.path:
        sys.path.insert(0, _p)

from contextlib import ExitStack

import ml_dtypes
import numpy as np

import concourse.bass as bass
import concourse.bacc as bacc_mod
import concourse.mybir as mybir
import concourse.tile as tile
from concourse.masks import make_identity

F32 = mybir.dt.float32
BF16 = mybir.dt.bfloat16
BF16NP = ml_dtypes.bfloat16

# Problem constants (hardcoded per harness contract)
B, N, C, CV, H = 4, 2048, 768, 512, 8
DH, DV = C // H, CV // H          # 96, 64
NH = 4                            # heads per core
NDO = NH * DH                     # 384 projected q/k dims per core
NDV = NH * DV                     # 256 projected v dims per core
SCALE = float((256 // 8) ** (-0.5))
N_CORES = 8


def build_nc(NT: int = N):
    """Build the per-core Bass program. NT = token count (param for small sims)."""
    NJT = NT // 128               # j tiles
    NIT = NT // 128               # i tiles
    assert NT % 512 == 0
    ICH = 1024 if NT % 1024 == 0 else 512   # exp chunk width

    nc = bacc_mod.Bacc()
    qT = nc.declare_dram_parameter("qT", [C, NT], BF16, isOutput=False)
    kT = nc.declare_dram_parameter("kT", [C, NT], BF16, isOutput=False)
    vT = nc.declare_dram_parameter("vT", [CV, NT], BF16, isOutput=False)
    mT = nc.declare_dram_parameter("mT", [NT, NT], BF16, isOutput=False)
    wqT = nc.declare_dram_parameter("wqT", [C, NDO], BF16, isOutput=False)
    wkT = nc.declare_dram_parameter("wkT", [C, NDO], BF16, isOutput=False)
    wvT = nc.declare_dram_parameter("wvT", [CV, NDV], BF16, isOutput=False)
    out = nc.declare_dram_parameter("out", [NT, NDV], F32, isOutput=True)

    NCT = C // 128                # 6 c tiles
    NVT = CV // 128               # 4 cv tiles

    with ExitStack() as top:
        tc = top.enter_context(tile.TileContext(nc))
        persist = top.enter_context(tc.tile_pool(name="persist", bufs=1))

        # ---- masks (transposed) resident in SBUF ----
        mt_tiles = []
        for jt in range(NJT):
            t = persist.tile([128, NT], BF16, tag=f"mt{jt}", name=f"mt{jt}")
            nc.sync.dma_start(out=t, in_=mT[jt * 128:(jt + 1) * 128, :])
            mt_tiles.append(t)

        # summ8[i] = 8 * sum_j m[i, j]; filled during head 0 via a ones-row
        # matmul over mT accumulated into ut_ps rows 96 (see below).
        summ8 = persist.tile([128, NIT], F32, tag="summ8", name="summ8")

        # ---- projections ----
        qpt = [persist.tile([DH, NT], BF16, tag=f"qpt{h}", name=f"qpt{h}") for h in range(NH)]
        kpt = [persist.tile([DH, NT], BF16, tag=f"kpt{h}", name=f"kpt{h}") for h in range(NH)]
        vp = persist.tile([128, NJT, NDV], BF16, tag="vp", name="vp")

        with ExitStack() as projctx:
            qkv_pool = projctx.enter_context(tc.tile_pool(name="qkv", bufs=1))
            w_pool = projctx.enter_context(tc.tile_pool(name="w", bufs=1))
            ppsum = projctx.enter_context(
                tc.tile_pool(name="ppsum", bufs=4, space="PSUM"))

            def load_tiles(dram, n_tiles, width, tagp):
                ts = []
                for i in range(n_tiles):
                    t = qkv_pool.tile([128, width], BF16, tag=f"{tagp}{i}", name=f"{tagp}{i}")
                    nc.sync.dma_start(out=t, in_=dram[i * 128:(i + 1) * 128, :])
                    ts.append(t)
                return ts

            qts = load_tiles(qT, NCT, NT, "q")
            kts = load_tiles(kT, NCT, NT, "k")
            vts = load_tiles(vT, NVT, NT, "v")
            wqts = []
            wkts = []
            wvts = []
            for i in range(NCT):
                t = w_pool.tile([128, NDO], BF16, tag=f"wq{i}", name=f"wq{i}")
                nc.sync.dma_start(out=t, in_=wqT[i * 128:(i + 1) * 128, :])
                wqts.append(t)
                t = w_pool.tile([128, NDO], BF16, tag=f"wk{i}", name=f"wk{i}")
                nc.sync.dma_start(out=t, in_=wkT[i * 128:(i + 1) * 128, :])
                wkts.append(t)
            for i in range(NVT):
                t = w_pool.tile([128, NDV], BF16, tag=f"wv{i}", name=f"wv{i}")
                nc.sync.dma_start(out=t, in_=wvT[i * 128:(i + 1) * 128, :])
                wvts.append(t)

            # QpT/KpT: out[dh, tok-chunk] = W_slice @ x.T
            for h in range(NH):
                for dst, wts, xts in ((qpt, wqts, qts), (kpt, wkts, kts)):
                    for ch in range(NT // 512):
                        ps = ppsum.tile([DH, 512], F32, tag="pp", name="pp")
                        for ci in range(NCT):
                            nc.tensor.matmul(
                                ps,
                                lhsT=wts[ci][:, h * DH:(h + 1) * DH],
                                rhs=xts[ci][:, ch * 512:(ch + 1) * 512],
                                start=(ci == 0), stop=(ci == NCT - 1),
                            )
                        nc.vector.tensor_copy(
                            out=dst[h][:, ch * 512:(ch + 1) * 512], in_=ps)

            # Vp natural: out[tok-tile, dv_all]
            for jt in range(NJT):
                ps = ppsum.tile([128, NDV], F32, tag="pv", name="pv")
                for ci in range(NVT):
                    nc.tensor.matmul(
                        ps,
                        lhsT=vts[ci][:, jt * 128:(jt + 1) * 128],
                        rhs=wvts[ci],
                        start=(ci == 0), stop=(ci == NVT - 1),
                    )
                nc.vector.tensor_copy(out=vp[:, jt, :], in_=ps)

        # ---- attention ----
        ones = persist.tile([128, 1], BF16, tag="ones", name="ones")
        nc.vector.memset(ones, 1.0)
        ident = persist.tile([128, 128], F32, tag="ident", name="ident")
        make_identity(nc, ident)
        x_sb = [persist.tile([128, NDV], F32, tag=f"x{it}", name=f"x{it}") for it in range(NIT)]

        spsum = top.enter_context(tc.tile_pool(name="spsum", bufs=2, space="PSUM"))
        utpsum = top.enter_context(tc.tile_pool(name="utpsum", bufs=1, space="PSUM"))
        streams = top.enter_context(tc.tile_pool(name="streams", bufs=3))
        utsb_pool = top.enter_context(tc.tile_pool(name="utsb", bufs=2))
        small = top.enter_context(tc.tile_pool(name="small", bufs=4))

        # ---- summ8 = 8 * row-sums of masks, via ones-matmul over mT ----
        summr = persist.tile([1, NT], F32, tag="summr", name="summr")
        for half in range(NT // ICH):
            sm_ps = spsum.tile([1, ICH], F32, tag="s", name="sm_ps")
            for jt in range(NJT):
                for q2 in range(ICH // 512):
                    nc.tensor.matmul(
                        sm_ps[:, q2 * 512:(q2 + 1) * 512],
                        lhsT=ones,
                        rhs=mt_tiles[jt][:, half * ICH + q2 * 512:
                                         half * ICH + (q2 + 1) * 512],
                        start=(jt == 0), stop=(jt == NJT - 1),
                        skip_group_check=True,
                    )
            nc.vector.tensor_copy(
                out=summr[:, half * ICH:(half + 1) * ICH], in_=sm_ps)
        sumn_ps = spsum.tile([128, NIT], F32, tag="s", name="sumn_ps")
        for it in range(NIT):
            nc.tensor.transpose(
                out=sumn_ps[:, it:it + 1],
                in_=summr[:, it * 128:(it + 1) * 128],
                identity=ident[0:1, 0:1],
            )
        nc.vector.tensor_scalar_mul(summ8, sumn_ps, float(H))

        for h in range(NH):
            ut_ps = utpsum.tile([128, NT], F32, tag="ut", name="ut")
            for jt in range(NJT):
                expst = streams.tile([128, NT], BF16, tag="expst", name="expst")
                for ih in range(NT // ICH):
                    s_ps = spsum.tile([128, ICH], F32, tag="s", name="s_ps")
                    for q2 in range(ICH // 512):
                        nc.tensor.matmul(
                            s_ps[:, q2 * 512:(q2 + 1) * 512],
                            lhsT=kpt[h][:, jt * 128:(jt + 1) * 128],
                            rhs=qpt[h][:, ih * ICH + q2 * 512: ih * ICH + (q2 + 1) * 512],
                            start=True, stop=True,
                        )
                    nc.scalar.activation(
                        out=expst[:, ih * ICH:(ih + 1) * ICH], in_=s_ps,
                        func=mybir.ActivationFunctionType.Exp, scale=SCALE,
                    )
                bsb = streams.tile([128, NT], BF16, tag="b", name="bsb")
                nc.vector.tensor_tensor(
                    out=bsb, in0=expst, in1=mt_tiles[jt], op=mybir.AluOpType.mult)
                first, last = (jt == 0), (jt == NJT - 1)
                for ic in range(NT // 512):
                    sl = slice(ic * 512, (ic + 1) * 512)
                    nc.tensor.matmul(
                        ut_ps[0:DV, sl],
                        lhsT=vp[:, jt, h * DV:(h + 1) * DV],
                        rhs=bsb[:, sl],
                        start=first, stop=last, skip_group_check=True,
                    )
                    nc.tensor.matmul(
                        ut_ps[DV:DV + 1, sl],
                        lhsT=ones,
                        rhs=expst[:, sl],
                        start=first, stop=last, skip_group_check=True,
                    )


            # head epilogue: transpose U.T -> natural, normalize, store
            ut_sb = utsb_pool.tile([DV + 1, NT], F32, tag="utsb", name="utsb")
            nc.vector.tensor_copy(out=ut_sb, in_=ut_ps[0:DV + 1, :])
            ng = max(NIT // 4, 1)
            gsz = NIT // ng
            for g in range(ng):
                un_ps = spsum.tile([128, gsz, DV + 1], F32, tag="s", name="un_ps")
                for t in range(gsz):
                    it = g * gsz + t
                    nc.tensor.transpose(
                        out=un_ps[:, t, :],
                        in_=ut_sb[:, it * 128:(it + 1) * 128],
                        identity=ident[0:DV + 1, 0:DV + 1],
                    )
                den = small.tile([128, gsz], F32, tag="den", name="den")
                rec = small.tile([128, gsz], F32, tag="rec", name="rec")
                nc.vector.tensor_tensor(
                    out=den, in0=un_ps[:, :, DV],
                    in1=summ8[:, g * gsz:(g + 1) * gsz], op=mybir.AluOpType.mult)
                nc.vector.reciprocal(rec, den)
                for t in range(gsz):
                    it = g * gsz + t
                    nc.vector.tensor_scalar_mul(
                        x_sb[it][:, h * DV:(h + 1) * DV],
                        un_ps[:, t, 0:DV],
                        rec[:, t:t + 1],
                    )

        for it in range(NIT):
            nc.sync.dma_start(out=out[it * 128:(it + 1) * 128, :], in_=x_sb[it])

    nc.finalize()
    return nc


_NC_CACHE: dict = {}


def get_nc(NT: int = N):
    if NT not in _NC_CACHE:
        _NC_CACHE[NT] = build_nc(NT)
    return _NC_CACHE[NT]


def make_in_maps(q, k, v, masks, Wq, Wk, Wv):
    """Host-side shard + layout prep. Returns per-core input dicts."""

    def bf(x):
        return np.ascontiguousarray(x.astype(np.float32).astype(BF16NP))

    in_maps = []
    for c in range(N_CORES):
        b, hg = c // 2, c % 2
        in_maps.append({
            "qT": bf(q[b].T),
            "kT": bf(k[b].T),
            "vT": bf(v[b].T),
            "mT": bf(masks[b].T),
            "wqT": bf(Wq[hg * NDO:(hg + 1) * NDO, :].T),
            "wkT": bf(Wk[hg * NDO:(hg + 1) * NDO, :].T),
            "wvT": bf(Wv[hg * NDV:(hg + 1) * NDV, :].T),
        })
    return in_maps


def kernel(q, k, v, masks, Wq, Wk, Wv, **_unused):
    from concourse.bass_utils import run_bass_kernel_spmd

    q, k, v, masks = (np.asarray(x) for x in (q, k, v, masks))
    Wq, Wk, Wv = (np.asarray(x) for x in (Wq, Wk, Wv))

    nc = get_nc(N)
    in_maps = make_in_maps(q, k, v, masks, Wq, Wk, Wv)
    res = run_bass_kernel_spmd(nc, in_maps, core_ids=list(range(N_CORES))).results

    full = np.empty((B, N, CV), np.float32)
    for c in range(N_CORES):
        b, hg = c // 2, c % 2
        full[b][:, hg * NDV:(hg + 1) * NDV] = res[c]["out"]
    return full
